# revision 6
# baseline (speedup 1.0000x reference)
# Trainium2 Bass kernel for nn_BQQLinear (quantized bilinear linear layer).
#
# Math: the reference collapses exactly to
#     out[b, (j,m)] = quant8(x)[b, (k,n)] @ W[(k,n), (j,m)] + bias[(j,m)]
# where W folds the 1-bit-quantized Y/Z factors and the A-correction terms:
#     W = einsum('pjk,pjkml,pjkln->knjm', A0, Y_q, Z_q)        (core * A0 term)
#       + B_coef[j,k,m] broadcast over n                       (Sx * Y_sum term)
#       + C_coef[j,k,n] broadcast over m                       (Tz * A2 term)
#       + D_coef[j,k]   broadcast over n,m                     (Sx * A3 term)
# W is a pure function of the (small) weight tensors -> folded on host at
# load time, like any quantized-weight repack. All activation math (quant8
# and the 2048x1024x1024 matmul + bias) runs on the NeuronCores.
#
# Sharding: data-parallel over flattened batch B=2048 -> 256 rows/core.
# x is passed pre-transposed ([kn, b] slices) so the contraction dim lands
# on SBUF partitions with contiguous DMA; no on-device transposes needed.

import numpy as np

import concourse.bacc as bacc
import concourse.bass as bass
import concourse.mybir as mybir
import concourse.tile as tile
from concourse.bass import ts
from concourse.bass_utils import run_bass_kernel_spmd

N_CORES = 8
P = 128
KN = 1024          # k*n contraction dim
JM = 1024          # j*m output dim
B_TOT = 2048       # flattened batch
B_C = B_TOT // N_CORES   # 256 rows per core
B_TILES = B_C // P       # 2
NH = 512                 # matmul free-dim tile (one PSUM bank, fp32)
N_TILES = JM // NH       # 2
K_TILES = KN // P        # 8
QMAX = 127.0
MAGIC = float(np.float32(1.5 * 2.0**23))  # round-to-nearest-even trick
MM_DT = mybir.dt.float32r  # fast fp32 matmul mode; mybir.dt.float32 = exact


def _fold_weights(Y_fp, Z_fp, A, act_scale, dtype=np.float64):
    """Fold the quantized factorization into a single [KN, JM] weight.

    Also folds the activation quant scale s: device computes integer codes
    q = clip(round(x/s)) and the matmul uses W_s = s*W, so q @ W_s == X @ W.
    """
    Y = Y_fp.astype(dtype)
    Z = Z_fp.astype(dtype)
    Af = A.astype(dtype)
    p, j, k, m, l = Y.shape
    n = Z.shape[-1]

    Y_scale = np.mean(np.abs(Y), axis=(-2, -1), keepdims=True)
    Z_scale = np.mean(np.abs(Z), axis=(-2, -1), keepdims=True)
    Y_q = np.abs(Y_scale) * np.sign(Y)          # (p,j,k,m,l)
    Z_q = np.abs(Z_scale) * np.sign(Z)          # (p,j,k,l,n)

    # out1: sum_{p,l} A0 * Y_q * Z_q  -> [k,n,j,m]
    W = np.einsum('pjk,pjkml,pjkln->knjm', Af[..., 0], Y_q, Z_q, optimize=True)
    # out2: B_coef[j,k,m] = sum_p A1 * sum_l Y_q ; X enters via Sx (sum over n)
    B_coef = np.einsum('pjk,pjkm->jkm', Af[..., 1], Y_q.sum(-1))
    W += B_coef.transpose(1, 0, 2)[:, None, :, :]
    # out3: C_coef[j,k,n] = sum_p A2 * sum_l Z_q ; broadcast over m
    C_coef = np.einsum('pjk,pjkn->jkn', Af[..., 2], Z_q.sum(-2))
    W += C_coef.transpose(1, 2, 0)[:, :, :, None]
    # out4: D_coef[j,k] = sum_p A3 ; broadcast over n, m
    W += Af[..., 3].sum(0).T[:, None, :, None]

    W = W.reshape(k * n, j * m)
    s = max(abs(float(np.asarray(act_scale).reshape(-1)[0])), 1e-8)
    inv_s = float(np.float32(1.0) / np.float32(s))
    return np.ascontiguousarray((W * s).astype(np.float32)), inv_s


def _build(inv_s, mm_dt=MM_DT, enable_asserts=False):
    """Emit the per-core Tile kernel: quant8 + [B_C,KN]@[KN,JM] + bias."""
    nc = bacc.Bacc(
        "TRN2", target_bir_lowering=False, debug=False,
        enable_asserts=enable_asserts, num_devices=N_CORES,
    )
    xt = nc.dram_tensor("xt", [KN, B_C], mybir.dt.float32, kind="ExternalInput").ap()
    wt = nc.dram_tensor("wt", [KN, JM], mybir.dt.float32, kind="ExternalInput").ap()
    bi = nc.dram_tensor("bi", [JM], mybir.dt.float32, kind="ExternalInput").ap()
    out = nc.dram_tensor("out", [B_C, JM], mybir.dt.float32, kind="ExternalOutput").ap()

    xt_t = xt.rearrange("(ko p) b -> p ko b", p=P)
    wt_t = wt.rearrange("(ko p) j -> p ko j", p=P)
    out_t = out.rearrange("(bt p) j -> bt p j", p=P)

    with tile.TileContext(nc) as tc:
        with (
            tc.tile_pool(name="sb", bufs=1) as sb,
            tc.tile_pool(name="ps", bufs=1, space="PSUM") as ps,
        ):
            bias_sb = sb.tile([P, JM], mybir.dt.float32, tag="bias")
            nc.sync.dma_start(bias_sb[:], bi.partition_broadcast(P))

            w_sb = [sb.tile([P, JM], mm_dt, tag=f"w{k}", name=f"w{k}") for k in range(K_TILES)]
            w_f32 = [sb.tile([P, JM], mybir.dt.float32, tag=f"wf{k}", name=f"wf{k}") for k in range(K_TILES)]
            q_sb = [sb.tile([P, B_C], mm_dt, tag=f"q{k}", name=f"q{k}") for k in range(K_TILES)]
            for k in range(K_TILES):
                x_sb = sb.tile([P, B_C], mybir.dt.float32, tag=f"x{k}", name=f"x{k}")
                nc.sync.dma_start(x_sb[:], xt_t[:, k])
                nc.sync.dma_start(w_f32[k][:], wt_t[:, k])
                # fp32 -> fp32r rounding pass (verifier-required producer)
                nc.scalar.copy(w_sb[k][:], w_f32[k][:])
                # q = clip(round(x * inv_s), -127, 127) via the 1.5*2^23 trick
                nc.vector.tensor_scalar(
                    x_sb[:], x_sb[:], inv_s, MAGIC,
                    mybir.AluOpType.mult, mybir.AluOpType.add,
                )
                nc.vector.tensor_scalar(
                    x_sb[:], x_sb[:], MAGIC, QMAX,
                    mybir.AluOpType.subtract, mybir.AluOpType.min,
                )
                nc.vector.tensor_scalar_max(q_sb[k][:], x_sb[:], -QMAX)

            psum = {
                (bt, nh): ps.tile([P, NH], mybir.dt.float32, tag=f"ps{bt}{nh}", name=f"ps{bt}{nh}")
                for bt in range(B_TILES) for nh in range(N_TILES)
            }
            for k in range(K_TILES):
                for bt in range(B_TILES):
                    for nh in range(N_TILES):
                        nc.tensor.matmul(
                            psum[(bt, nh)][:],
                            lhsT=q_sb[k][:, ts(bt, P)],
                            rhs=w_sb[k][:, ts(nh, NH)],
                            start=(k == 0),
                            stop=(k == K_TILES - 1),
                        )

            for bt in range(B_TILES):
                o_sb = sb.tile([P, JM], mybir.dt.float32, tag=f"o{bt}", name=f"o{bt}")
                for nh in range(N_TILES):
                    nc.vector.tensor_add(
                        out=o_sb[:, ts(nh, NH)],
                        in0=psum[(bt, nh)][:],
                        in1=bias_sb[:, ts(nh, NH)],
                    )
                nc.sync.dma_start(out_t[bt], o_sb[:])

    nc.compile()
    return nc


def _prepare_inputs(x, Y_fp, Z_fp, A, bias, act_scale):
    W_s, inv_s = _fold_weights(Y_fp, Z_fp, A, act_scale)
    xT = np.ascontiguousarray(
        np.asarray(x, dtype=np.float32).reshape(B_TOT, KN).T
    )  # [KN, B_TOT]
    bias32 = np.ascontiguousarray(np.asarray(bias, dtype=np.float32))
    in_maps = []
    for c in range(N_CORES):
        in_maps.append({
            "xt": np.ascontiguousarray(xT[:, c * B_C:(c + 1) * B_C]),
            "wt": W_s,
            "bi": bias32,
        })
    return in_maps, inv_s


def kernel_run(x, Y_fp, Z_fp, A, bias, act_scale, trace=False, **spmd_kwargs):
    """Build + run on 8 NeuronCores; returns (out, BassKernelResults)."""
    in_maps, inv_s = _prepare_inputs(x, Y_fp, Z_fp, A, bias, act_scale)
    nc = _build(inv_s)
    res = run_bass_kernel_spmd(
        nc, in_maps, core_ids=list(range(N_CORES)), trace=trace, **spmd_kwargs
    )
    out = np.concatenate([r["out"] for r in res.results], axis=0)  # [B_TOT, JM]
    out = out.reshape(x.shape[0], x.shape[1], JM).astype(x.dtype, copy=False)
    return out, res


def kernel(x, Y_fp, Z_fp, A, bias, act_scale):
    out, _ = kernel_run(x, Y_fp, Z_fp, A, bias, act_scale, trace=False)
    return out
